# revision 26
# baseline (speedup 1.0000x reference)
"""BertSelfAttention Trainium2 kernel (8-core SPMD, head-parallel).

Sharding: 16 heads / 8 cores = 2 heads per core (tensor-parallel QKV).
Each core computes q/k/v projections for its 128 output dims over the
full [B*S, D] input, then attention for its 2 heads over all batches.
No collectives: host slices W/b per core and concatenates outputs.

Host prep (one-time, outside the timed device program): hidden is
transposed to [D, B*S] and cast to bf16 so the device needs no PE
transposes for the projections; weights are passed pre-transposed
[D, 128] bf16 per core.

Per-core dataflow:
  hT chunk [di, tok] <-- direct DMA from host-transposed hidT
  qT/kT = wT.T @ hT   (dims on partitions, tokens free)   [128, B*S] bf16
  v     = (vT transposed back) [tok, dims] + ones column  (for rowsum)
  per (b, q-half):
    for each key tile i (128 keys):
      sp[128, 2, 1024] bf16 PSUM  = two row-tiled K=64 matmuls
                                    (head0 rows 0-63, head1 rows 64-127)
      e = exp(sp/8 + mask_i)      one ACT instr, N=2048
      ctx_ps[h] (+= [v|1].T @ e)  accumulated over i, M=65 (ctx + rowsum)
    tail (deferred): transpose ctxT, divide by rowsum, DMA out
"""

import sys

sys.path.insert(0, "/opt/trn_rl_repo")

from contextlib import ExitStack

import numpy as np

import concourse.bass as bass
import concourse.bacc as bacc
import concourse.mybir as mybir
import concourse.tile as tile
from concourse.masks import make_identity

D = 1024
HD = 64
NCORES = 8
HPC = 2            # heads per core
DPC = HPC * HD     # 128 output dims per core

FP = mybir.dt.float32
BF = mybir.dt.bfloat16
AF = mybir.ActivationFunctionType
ALU = mybir.AluOpType


def build_core_program(b: int, s: int, repeat: int = 1,
                       io_lite: bool = False, parts: str = "both",
                       interleave: bool = True):
    """Build the per-core Bass program (same program on all 8 cores).

    io_lite=True builds a timing-only variant: all real I/O tensors are
    Internal DRAM (no per-call transfer over the axon tunnel, whose
    ~8.5 GB/s would otherwise hide device exec time); a tiny in/out pair
    keeps the NEFF I/O contract alive. Compute stream is identical.
    """
    assert s % 128 == 0 and D == 1024
    bs = b * s
    n_sk = s // 128          # key tiles per batch
    ntt = bs // 128          # token tiles total
    CHUNK = min(512, bs)     # phase-1 token chunk
    TPC = CHUNK // 128
    n_chunks = bs // CHUNK
    SQH = min(512, s)        # phase-2 query span (1 PSUM bank per tile)
    n_half = s // SQH
    NJ = SQH // 128          # query tiles per span

    nc = bacc.Bacc("TRN2", target_bir_lowering=False, debug=False)

    kio = "Internal" if io_lite else "ExternalInput"
    koo = "Internal" if io_lite else "ExternalOutput"
    hidT = nc.dram_tensor("hidT", [D, bs], BF, kind=kio)
    msk = nc.dram_tensor("mask", [b, s], FP, kind=kio)
    wqT = nc.dram_tensor("wqT", [D, DPC], BF, kind=kio)
    wkT = nc.dram_tensor("wkT", [D, DPC], BF, kind=kio)
    wvT = nc.dram_tensor("wvT", [D, DPC], BF, kind=kio)
    bq = nc.dram_tensor("bq", [DPC], FP, kind=kio)
    bk = nc.dram_tensor("bk", [DPC], FP, kind=kio)
    bv = nc.dram_tensor("bv", [DPC], FP, kind=kio)
    out = nc.dram_tensor("out", [bs, DPC], FP, kind=koo)
    if io_lite:
        tin = nc.dram_tensor("tin", [1, 4], FP, kind="ExternalInput")
        tout = nc.dram_tensor("tout", [1, 4], FP, kind="ExternalOutput")

    with tile.TileContext(nc) as tc, ExitStack() as ctx:
        singles = ctx.enter_context(tc.tile_pool(name="singles", bufs=1))

        ident_bf = singles.tile([128, 128], BF, tag="ident_bf")
        make_identity(nc, ident_bf)
        ident_f32 = singles.tile([128, 128], FP, tag="ident_f32")
        make_identity(nc, ident_f32)

        # ---- weights (pre-transposed on host): [D, 128] -> [128, 8, 128]
        wT_sbs = []
        bias_sbs = []
        for widx, (wd, bd) in enumerate(((wqT, bq), (wkT, bk), (wvT, bv))):
            wT = singles.tile([128, 8, DPC], BF, tag=f"wT{widx}")
            nc.sync.dma_start(
                out=wT, in_=wd[:, :].rearrange("(k p) o -> p k o", p=128)
            )
            wT_sbs.append(wT)
            bsb = singles.tile([128, 1], FP, tag=f"bias{widx}")
            nc.sync.dma_start(
                out=bsb, in_=bd[:].rearrange("(p o) -> p o", o=1)
            )
            bias_sbs.append(bsb)

        # ---- mask: [b, s] -> mask_sb[p, bb*n_sk + i] = mask[bb, i*128+p]
        mask_sb = singles.tile([128, b * n_sk], FP, tag="mask_sb")
        with tc.tile_pool(name="prep_ps", bufs=2, space="PSUM") as pprep, \
             tc.tile_pool(name="prep_sb", bufs=2) as psb:
            for bb in range(b):
                m_nat = psb.tile([n_sk, 128], FP, tag="m_nat")
                nc.sync.dma_start(
                    out=m_nat,
                    in_=msk[bb, :].rearrange("(j p) -> j p", p=128),
                )
                pm = pprep.tile([128, n_sk], FP, tag="pm")
                nc.tensor.transpose(pm, m_nat, ident_f32[0:n_sk, 0:n_sk])
                nc.vector.tensor_copy(
                    out=mask_sb[:, bb * n_sk:(bb + 1) * n_sk], in_=pm
                )

        # ---- persistent qkv tensors ----
        qT_sb = singles.tile([128, bs], BF, tag="qT_sb")
        kT_sb = singles.tile([128, bs], BF, tag="kT_sb")
        # v with ones columns: [tok_in_tile, tile*(64+1+64+1)]
        v_sb = singles.tile([128, ntt, 2, 65], BF, tag="v_sb")
        nc.vector.memset(v_sb[:, :, :, 64:65], 1.0)
        if parts in ("p2", "p2s", "p2se"):
            # timing-only build without phase 1: give the qkv tensors a
            # defining write so Tile can allocate them
            nc.vector.memset(qT_sb, 0.01)
            nc.vector.memset(kT_sb, 0.01)
            nc.vector.memset(v_sb[:, :, :, 0:64], 0.01)

        for _rep in range(repeat):
            # ==== phase 1 (projections) interleaved with phase 2
            # (attention) per batch: attention for batch bb only needs
            # batch bb's projections, so p2(bb)'s ACT-bound exp stream
            # overlaps p1(bb+1)'s PE/DMA work ====
            with tc.tile_pool(name="p1_h", bufs=4) as p1h, \
                 tc.tile_pool(name="p1_sb", bufs=3) as p1sb, \
                 tc.tile_pool(name="p1_ps", bufs=1, space="PSUM") as p1ps, \
                 tc.tile_pool(name="p1_pv", bufs=1, space="PSUM") as p1pv, \
                 tc.tile_pool(name="p2_sp", bufs=2, space="PSUM") as p2sp, \
                 tc.tile_pool(name="p2_ctx", bufs=1, space="PSUM") as p2ctx, \
                 tc.tile_pool(name="p2_e", bufs=3) as p2e, \
                 tc.tile_pool(name="p2_tail", bufs=3) as p2tail:

                def p1_chunk_items(c):
                    """Yield phase-1 work for chunk c as small closures so
                    the span loop can drip-feed them into the PE stream."""
                    c0 = c * CHUNK
                    hT = p1h.tile([128, 8, CHUNK], BF, tag="hT")
                    eng = nc.gpsimd if c % 2 == 0 else nc.sync

                    def dma():
                        eng.dma_start(
                            out=hT,
                            in_=hidT[:, c0:c0 + CHUNK].rearrange(
                                "(k p) t -> p k t", p=128
                            ),
                        )
                    yield dma

                    state = {}

                    def mm_pair(widx, kk2):
                        def go():
                            if kk2 == 0:
                                state[widx] = p1ps.tile(
                                    [128, CHUNK], FP, name=f"ps{widx}",
                                    tag="ps_qkv")
                            ps = state[widx]
                            for kk in (kk2 * 2, kk2 * 2 + 1):
                                nc.tensor.matmul(
                                    ps,
                                    wT_sbs[widx][:, kk, :],
                                    hT[:, kk, :],
                                    start=(kk == 0),
                                    stop=(kk == 7),
                                )
                        return go

                    def finish(widx):
                        def go():
                            ps = state[widx]
                            if widx == 0:
                                nc.vector.tensor_scalar(
                                    qT_sb[:, c0:c0 + CHUNK], ps,
                                    bias_sbs[0], None, ALU.add,
                                )
                            elif widx == 1:
                                nc.vector.tensor_scalar(
                                    kT_sb[:, c0:c0 + CHUNK], ps,
                                    bias_sbs[1], None, ALU.add,
                                )
                            else:
                                vt_stage = p1sb.tile(
                                    [128, CHUNK], BF, name="vt_stage",
                                    tag="vt_stage")
                                nc.vector.tensor_scalar(
                                    vt_stage, ps, bias_sbs[2], None,
                                    ALU.add,
                                )
                                state["vt"] = vt_stage
                        return go

                    for widx in range(3):
                        for kk2 in range(4):
                            yield mm_pair(widx, kk2)
                        yield finish(widx)

                    def vtrans(j2):
                        def go():
                            if j2 == 0:
                                state["pv"] = p1pv.tile(
                                    [128, CHUNK], BF, name="pv", tag="pv")
                            pv, vt_stage = state["pv"], state["vt"]
                            for j in (j2 * 2, j2 * 2 + 1):
                                nc.tensor.transpose(
                                    pv[:, j * 128:(j + 1) * 128],
                                    vt_stage[:, j * 128:(j + 1) * 128],
                                    ident_bf,
                                )
                        return go

                    for j2 in range(TPC // 2):
                        yield vtrans(j2)

                    def vcopy():
                        tt0 = c0 // 128
                        nc.vector.tensor_copy(
                            out=v_sb[:, tt0:tt0 + TPC, :, 0:64],
                            in_=state["pv"].rearrange(
                                "p (t h x) -> p t h x", t=TPC, h=2
                            ),
                        )
                    yield vcopy

                def emit_p1_chunk(c):
                    for item in p1_chunk_items(c):
                        item()

                def emit_tail(cnTs, q0):
                    osb = p2tail.tile([128, NJ, DPC], FP, name="osb",
                                      tag="osb")
                    for h in range(HPC):
                        cnT = cnTs[h]
                        ps2 = p2ctx.tile([128, NJ, 128], FP,
                                         name=f"ps2_{h}", tag=f"ctx{h}")
                        for j in range(NJ):
                            nc.tensor.transpose(
                                ps2[:, j, 0:65],
                                cnT[:, j * 128:(j + 1) * 128],
                                ident_f32[0:65, 0:65],
                            )
                        rcp = p2tail.tile(
                            [128, NJ, 1], FP, name=f"rcp{h}", tag=f"rcp{h}"
                        )
                        nc.vector.reciprocal(out=rcp, in_=ps2[:, :, 64:65])
                        rbc = bass.AP(
                            tensor=rcp.tensor,
                            offset=rcp.offset,
                            ap=[rcp.ap[0], rcp.ap[1], [0, 64]],
                        )
                        nc.vector.tensor_mul(
                            osb[:, :, h * 64:(h + 1) * 64],
                            ps2[:, :, 0:64],
                            rbc,
                        )
                    nc.sync.dma_start(
                        out=out[q0:q0 + SQH, :].rearrange(
                            "(j p) dd -> p j dd", p=128
                        ),
                        in_=osb,
                    )

                CW = min(512, SQH)       # matmul free-dim chunk
                NCH = SQH // CW
                cpb = n_chunks // b      # phase-1 chunks per batch

                def emit_p2_span(bb, hf, pending, feed):
                    do_exp = parts != "p2s"
                    do_ctx = parts not in ("p2s", "p2se")
                    q0 = bb * s + hf * SQH
                    ctx_ps = [
                        p2ctx.tile([65, SQH], FP, name=f"ctx{h}",
                                   tag=f"ctx{h}")
                        for h in range(HPC)
                    ] if do_ctx else None
                    if pending:
                        emit_tail(*pending.pop())

                    def emit_scores_exp(i):
                        # one 2-bank PSUM tile holds both heads' scores;
                        # adjacent row-disjoint score MMs (head0 rows
                        # 0-63, head1 rows 64-127) overlap on the PE,
                        # then ONE wide exp covers both heads
                        sp = p2sp.tile([128, HPC, SQH], FP, name="sp",
                                       tag="sp")
                        for cc in range(NCH):
                            for h in range(HPC):
                                nc.tensor.matmul(
                                    sp[:, h, cc * CW:(cc + 1) * CW],
                                    kT_sb[h * 64:(h + 1) * 64,
                                          bb * s + i * 128:
                                          bb * s + (i + 1) * 128],
                                    qT_sb[h * 64:(h + 1) * 64,
                                          q0 + cc * CW:
                                          q0 + (cc + 1) * CW],
                                    start=True, stop=True,
                                )
                        es = None
                        if do_exp:
                            es = p2e.tile([128, HPC, SQH], BF, name="e",
                                          tag="e")
                            nc.scalar.activation(
                                out=es, in_=sp, func=AF.Exp,
                                scale=0.125,
                                bias=mask_sb[:, bb * n_sk + i:
                                             bb * n_sk + i + 1],
                            )
                        return es

                    def emit_ctx(i, es):
                        for h in range(HPC):
                            for cc in range(NCH):
                                nc.tensor.matmul(
                                    ctx_ps[h][:, cc * CW:(cc + 1) * CW],
                                    v_sb[:, bb * n_sk + i, h, :],
                                    es[:, h, cc * CW:(cc + 1) * CW],
                                    start=(i == 0), stop=(i == n_sk - 1),
                                )

                    # software pipeline: ctx MMs run one i behind the
                    # scores/exp pair so the in-order PE queue never
                    # waits on a fresh exp; p1 feed items fill PE slack
                    prev = None
                    for i in range(n_sk):
                        es = emit_scores_exp(i)
                        if prev is not None and do_ctx:
                            emit_ctx(i - 1, prev)
                        for _ in range(2):
                            if feed:
                                feed.popleft()()
                        prev = es
                    if not do_ctx:
                        return
                    emit_ctx(n_sk - 1, prev)

                    # evac ctx psum now (frees slot); defer rest
                    cnTs = []
                    for h in range(HPC):
                        cnT = p2tail.tile([65, SQH], FP, name=f"cnT{h}",
                                          tag=f"cnT{h}")
                        nc.vector.tensor_copy(out=cnT, in_=ctx_ps[h])
                        cnTs.append(cnT)
                    pending.append((cnTs, q0))

                from collections import deque
                pending = []
                feed = deque()
                if parts == "none":
                    pass
                elif interleave and parts == "both":
                    # batch 0's projections run standalone, then batch
                    # bb+1's projection items drip into bb's spans
                    for c in range(0, cpb):
                        emit_p1_chunk(c)
                    for bb in range(b):
                        if bb + 1 < b:
                            for c in range((bb + 1) * cpb,
                                           (bb + 2) * cpb):
                                feed.extend(p1_chunk_items(c))
                        for hf in range(n_half):
                            emit_p2_span(bb, hf, pending, feed)
                        while feed:
                            feed.popleft()()
                else:
                    if parts in ("p1", "both"):
                        for c in range(n_chunks):
                            emit_p1_chunk(c)
                    if parts != "p1":
                        for bb in range(b):
                            for hf in range(n_half):
                                emit_p2_span(bb, hf, pending, feed)
                if pending:
                    emit_tail(*pending.pop())

        if io_lite:
            tsb = singles.tile([1, 4], FP, tag="tiny")
            nc.sync.dma_start(out=tsb, in_=tin[:, :])
            nc.sync.dma_start(out=tout[:, :], in_=tsb)

    nc.compile()
    return nc


_CACHE = {}


def _get_program(b, s, repeat=1, io_lite=False, parts="both",
                 interleave=True):
    key = (b, s, repeat, io_lite, parts, interleave)
    if key not in _CACHE:
        _CACHE[key] = build_core_program(b, s, repeat, io_lite, parts,
                                         interleave)
    return _CACHE[key]


def kernel(hidden_states, attention_mask, Wq, bq, Wk, bk, Wv, bv):
    import ml_dtypes
    from concourse.bass_utils import run_bass_kernel_spmd

    hs = np.ascontiguousarray(np.asarray(hidden_states, dtype=np.float32))
    b, s, d = hs.shape
    assert d == D
    mk = np.ascontiguousarray(
        np.asarray(attention_mask, dtype=np.float32)
    ).reshape(b, s)
    ws = [np.asarray(w, dtype=np.float32) for w in (Wq, Wk, Wv)]
    bs_ = [np.asarray(x, dtype=np.float32) for x in (bq, bk, bv)]

    nc = _get_program(b, s)

    hidT = np.ascontiguousarray(
        hs.reshape(b * s, D).T
    ).astype(ml_dtypes.bfloat16)
    in_maps = []
    for c in range(NCORES):
        sl = slice(c * DPC, (c + 1) * DPC)
        in_maps.append({
            "hidT": hidT,
            "mask": mk,
            "wqT": np.ascontiguousarray(ws[0][sl].T).astype(
                ml_dtypes.bfloat16),
            "wkT": np.ascontiguousarray(ws[1][sl].T).astype(
                ml_dtypes.bfloat16),
            "wvT": np.ascontiguousarray(ws[2][sl].T).astype(
                ml_dtypes.bfloat16),
            "bq": np.ascontiguousarray(bs_[0][sl]),
            "bk": np.ascontiguousarray(bs_[1][sl]),
            "bv": np.ascontiguousarray(bs_[2][sl]),
        })

    res = run_bass_kernel_spmd(nc, in_maps, core_ids=list(range(NCORES)))
    parts = [res.results[c]["out"].reshape(b, s, DPC) for c in range(NCORES)]
    return np.concatenate(parts, axis=-1).astype(np.float32)
